# revision 16
# baseline (speedup 1.0000x reference)
"""Trainium2 Bass kernel for ragged subword mean pooling (nn_Bert).

Problem: out[b, j] = mean(bert_embedding[b, st_j:ed_j]) if (mask & ed>st) else 0
Shapes: bert_embedding [32, 1024, 768] f32, x_bert_offset [32, 768, 2] i32,
        x_mask [32, 768] i32 -> out [32, 768, 768] f32.

Strategy (pure data parallel, 4 batch rows per core on 8 cores):
  Spans are contiguous sorted segments, so per row the pooling is
  out = A.T @ E where A[s, j] = scale_j iff st_j <= s < ed_j
  (scale_j = valid/len folds the mean and mask directly into A).
  Each position s belongs to at most ONE word, so every A tile has at
  most one nonzero per partition row. The host ships just that
  (column, value) pair per position (~32KB/core) and the device
  reconstructs each [128, win] A window in a single fused DVE op
  against a constant column-index tile J:
      A[p, j] = (J[p, j] == idx_p) * val_p
  Only (m, k) tile pairs whose word/position ranges intersect are
  computed; the active-pair hull is derived on the host from the actual
  offsets (a superset is always correct since A is 0 outside).

This kernel is memory bound, so the optimization story is HBM bytes
and DMA/compute overlap:
  * All HBM I/O is fp16 (half of f32). PE contracts fp16 at full rate
    into f32 PSUM. Metadata (word indices <= 767, scales >= 1/1024) is
    fp16-exact; end-to-end rel err ~2e-4.
  * E is host-permuted so each row loads as one DMA of contiguous
    12 KB partition lines: E_in[r, p, k*D:+D] = E[r, k*128+p, :].
  * E loads are issued from the SP sequencer only; stores are issued
    from other engines, so a store waiting on compute never
    head-of-line-blocks the next row's E load (that stall serialized
    DMA behind compute, ~+15us).
  * Only ~64% of output words are valid (mask & nonempty). Stores go
    through indirect (scatter) DMA on the Pool queue with invalid rows
    pointed out-of-bounds -> skipped, saving ~36% of store traffic.
    The host zeroes invalid rows after gather. (scatter=False falls
    back to dense row stores in a host-unpermuted layout.)
  * PSUM drains alternate between the Act and DVE engines; A-builds
    are hoisted ahead of the row loop (they only depend on the tiny
    metadata DMA) so DVE drains never gate the next row's matmuls.
"""

import sys

if "/opt/trn_rl_repo" not in sys.path:
    sys.path.insert(0, "/opt/trn_rl_repo")

import numpy as np

B, S, W, D = 32, 1024, 768, 768
NCORES = 8
RPC = B // NCORES  # rows per core
KT = S // 128  # 8 k-tiles (positions)
MT = W // 128  # 6 m-tiles (words)
OOB = 1 << 20  # scatter index sentinel for invalid rows (skipped)

_CACHE = {}


def _active_pairs(st, ed):
    """Per row-slot r: hull of active k-tiles for each m-tile, and hull of
    active m-tiles for each k-tile, unioned over cores (the SPMD program is
    shared by all 8 cores). A superset only costs time, never correctness.
    """
    kl = []
    for r in range(RPC):
        per_m = []
        for m in range(MT):
            klo, khi = KT, 0
            for c in range(NCORES):
                b = c * RPC + r
                s0 = int(st[b, m * 128 : (m + 1) * 128].min())
                s1 = int(ed[b, m * 128 : (m + 1) * 128].max())
                if s1 > s0:
                    klo = min(klo, s0 // 128)
                    khi = max(khi, (s1 + 127) // 128)
            per_m.append((klo, khi) if khi > klo else None)
        kl.append(per_m)

    mw = []
    for r in range(RPC):
        per_k = []
        for k in range(KT):
            mlo, mhi = MT, 0
            for m in range(MT):
                if kl[r][m] and kl[r][m][0] <= k < kl[r][m][1]:
                    mlo = min(mlo, m)
                    mhi = max(mhi, m + 1)
            per_k.append((mlo, mhi) if mhi > mlo else None)
        mw.append(per_k)
    return kl, mw


def build_program(pairs, repeat=1, drain="both", io="ext", stage=3, nodma=False,
                  scatter=True, ebufs=5, abufs=33, psbufs=3, obufs=8):
    """Build the SPMD Bass program (one program, run on all 8 cores)."""
    import concourse.tile as tile
    from concourse import bacc, bass, mybir

    kl, mw = pairs
    f32 = mybir.dt.float32
    f16 = mybir.dt.float16
    i32 = mybir.dt.int32
    AF = mybir.ActivationFunctionType
    OP = mybir.AluOpType

    nc = bacc.Bacc(
        "TRN2", target_bir_lowering=False, debug=False, num_devices=NCORES
    )

    E_in = nc.dram_tensor("E_in", [RPC, 128, KT * D], f16, kind="ExternalInput").ap()
    # packed per (r, k): column 2*(r*KT+k) = one-hot column index within the
    # A window (or -1), column +1 = A value (scale of the word at that
    # position, 0 if masked/empty/uncovered)
    av_in = nc.dram_tensor("av_in", [128, RPC * KT * 2], f32, kind="ExternalInput").ap()
    if scatter:
        # scatter row index per (p, r*MT+m): r*W + m*128 + p, or OOB
        oi_in = nc.dram_tensor("oi_in", [128, RPC * MT], i32, kind="ExternalInput").ap()
        oshape = [RPC * W, D]
    else:
        oi_in = None
        oshape = [RPC, 128, MT * D]
    if io == "ext":
        out = nc.dram_tensor("out", oshape, f16, kind="ExternalOutput").ap()
        tok = None
    else:
        out = nc.dram_tensor("out_scratch", oshape, f16).ap()
        tok = nc.dram_tensor("tok", [128, 16], f16, kind="ExternalOutput").ap()
    outdma = not nodma

    def win(r, k):
        if mw[r][k] is None:
            return None
        mlo, mhi = mw[r][k]
        return mlo * 128, (mhi - mlo) * 128

    awidth = 128
    for r in range(RPC):
        for k in range(KT):
            if mw[r][k]:
                awidth = max(awidth, (mw[r][k][1] - mw[r][k][0]) * 128)

    with tile.TileContext(nc) as tc:
        with (
            tc.tile_pool(name="const", bufs=1) as cpool,
            tc.tile_pool(name="E", bufs=ebufs) as epool,
            tc.tile_pool(name="bc", bufs=2) as bcpool,
            tc.tile_pool(name="A", bufs=abufs) as apool,
            tc.tile_pool(name="outsb", bufs=obufs) as opool,
            tc.tile_pool(name="psum", bufs=psbufs, space="PSUM") as pspool,
        ):
            # constant column-index tile J[p, j] = j
            j_i = cpool.tile([128, awidth], i32)
            nc.gpsimd.iota(j_i[:], pattern=[[1, awidth]], base=0, channel_multiplier=0)
            j_f = cpool.tile([128, awidth], f16)
            nc.vector.tensor_copy(j_f[:], j_i[:])
            zeros = cpool.tile([128, D], f16)
            nc.vector.memset(zeros[:], 0.0)
            econst = avconst = None
            if nodma:
                econst = cpool.tile([128, KT * D], f16, tag="Ec")
                nc.vector.memset(econst[:], 0.5)
                avconst = cpool.tile([128, RPC * KT * 2], f32, tag="avc")
                nc.vector.memset(avconst[:], 3.0)

            def drain_to(oslice, src, i):
                eng = {"act": 0, "vector": 1}.get(drain, i % 2)
                if eng == 0:
                    nc.scalar.activation(oslice, src, AF.Copy)
                else:
                    nc.vector.tensor_copy(oslice, src)

            last_at = None
            for _ in range(repeat):
                if nodma:
                    av = avconst
                else:
                    av = bcpool.tile([128, RPC * KT * 2], f32, tag="av")
                    nc.sync.dma_start(av[:], av_in[:, :])
                    if scatter:
                        oi = bcpool.tile([128, RPC * MT], i32, tag="oi")
                        nc.sync.dma_start(oi[:], oi_in[:, :])

                # all A windows up front: they only depend on av, and
                # hoisting keeps DVE drains from gating later matmuls
                ak = {}
                for r in range(RPC):
                    for k in range(KT if stage >= 1 else 0):
                        w = win(r, k)
                        if w is None:
                            continue
                        j0, wd = w
                        c = (r * KT + k) * 2
                        at = apool.tile([128, awidth], f16, tag="A")
                        nc.vector.tensor_scalar(
                            at[:, :wd],
                            j_f[:, :wd],
                            av[:, c : c + 1],
                            av[:, c + 1 : c + 2],
                            OP.is_equal,
                            OP.mult,
                        )
                        ak[r, k] = (at, j0)
                        last_at = at

                ndrain = 0
                for r in range(RPC):
                    # whole E row in one contiguous DMA (12 KB per partition)
                    if nodma:
                        erow = econst
                    else:
                        erow = epool.tile([128, KT * D], f16, tag="E")
                        nc.sync.dma_start(erow[:], E_in[r])
                    et = [erow[:, k * D : (k + 1) * D] for k in range(KT)]

                    otile = None
                    if not scatter:
                        otile = opool.tile([128, MT * D], f16, tag="osb")
                    for m in range(MT):
                        active = kl[r][m] is not None and stage >= 2
                        if not active and scatter:
                            continue  # rows never written; host zeroes them
                        if scatter:
                            osb = opool.tile([128, D], f16, tag="osb")
                            oslice = osb[:]
                        else:
                            oslice = otile[:, m * D : (m + 1) * D]
                        if active:
                            klo, khi = kl[r][m]
                            ps = pspool.tile([128, D], f32, tag="ps")
                            for k in range(klo, khi):
                                at, j0 = ak[r, k]
                                lhsT = at[:, m * 128 - j0 : (m + 1) * 128 - j0]
                                for n0 in range(0, D, 512):
                                    n1 = min(n0 + 512, D)
                                    nc.tensor.matmul(
                                        ps[:, n0:n1],
                                        lhsT,
                                        et[k][:, n0:n1],
                                        start=(k == klo),
                                        stop=(k == khi - 1),
                                    )
                            if stage >= 3:
                                drain_to(oslice, ps[:], ndrain)
                            else:
                                drain_to(oslice, zeros[:], ndrain)
                        else:
                            drain_to(oslice, zeros[:], ndrain)
                        ndrain += 1
                        if outdma and scatter:
                            c = r * MT + m
                            nc.gpsimd.indirect_dma_start(
                                out=out[:],
                                out_offset=bass.IndirectOffsetOnAxis(
                                    ap=oi[:, c : c + 1], axis=0
                                ),
                                in_=oslice,
                                in_offset=None,
                                bounds_check=RPC * W - 1,
                                oob_is_err=False,
                            )
                    # dense store issued from Pool: it never blocks the SP
                    # load queue, and drains (Act/DVE) are never behind it
                    if outdma and not scatter:
                        nc.gpsimd.dma_start(out[r], otile[:])

            if tok is not None:
                if last_at is not None:
                    nc.sync.dma_start(tok[:], last_at[:, :16])
                else:
                    nc.sync.dma_start(tok[:], zeros[:, :16])

    nc.compile()
    return nc


def _prep(bert_embedding, x_bert_offset, x_mask, scatter=True):
    st = x_bert_offset[..., 0].astype(np.int64)
    ed = x_bert_offset[..., 1].astype(np.int64)
    length = ed - st
    valid = (x_mask > 0) & (length > 0)
    scale = np.where(
        valid, 1.0 / np.maximum(length, 1).astype(np.float64), 0.0
    ).astype(np.float32)
    st_ext = np.concatenate([st, ed[:, -1:]], axis=1)  # [B, W+1]

    # word index of each position (-1 if uncovered)
    word_of = np.full((B, S), -1, dtype=np.int64)
    s_idx = np.arange(S)
    for b in range(B):
        j = np.searchsorted(st_ext[b], s_idx, side="right") - 1
        ok = (j >= 0) & (j < W)
        word_of[b] = np.where(ok, j, -1)

    pairs = _active_pairs(st, ed)
    kl, mw = pairs

    # permuted fp16 E: E_perm[b, p, k*D:+D] = E[b, k*128+p, :]
    E = np.ascontiguousarray(
        np.asarray(bert_embedding, dtype=np.float16)
        .reshape(B, KT, 128, D)
        .transpose(0, 2, 1, 3)
        .reshape(B, 128, KT * D)
    )
    p_idx = np.arange(128)
    in_maps = []
    for c in range(NCORES):
        av = np.zeros((128, RPC * KT * 2), dtype=np.float32)
        oi = np.full((128, RPC * MT), OOB, dtype=np.int32)
        for r in range(RPC):
            b = c * RPC + r
            for k in range(KT):
                if mw[r][k] is None:
                    continue
                j0 = mw[r][k][0] * 128
                col = (r * KT + k) * 2
                s = k * 128 + np.arange(128)
                wj = word_of[b, s]
                covered = wj >= 0
                # window hull guarantees covered words lie inside [j0, j0+wd)
                av[:, col] = np.where(covered, wj - j0, -1).astype(np.float32)
                av[:, col + 1] = np.where(
                    covered, scale[b, np.clip(wj, 0, W - 1)], 0.0
                )
            for m in range(MT):
                j = m * 128 + p_idx
                oi[:, r * MT + m] = np.where(valid[b, j], r * W + j, OOB)
        im = {"E_in": E[c * RPC : (c + 1) * RPC], "av_in": av}
        if scatter:
            im["oi_in"] = oi
        in_maps.append(im)
    return pairs, in_maps


def kernel(bert_embedding, x_bert_offset, x_mask):
    from concourse.bass_utils import run_bass_kernel_spmd

    bert_embedding = np.asarray(bert_embedding, dtype=np.float32)
    x_bert_offset = np.asarray(x_bert_offset)
    x_mask = np.asarray(x_mask)
    pairs, in_maps = _prep(bert_embedding, x_bert_offset, x_mask)
    key = repr(pairs)
    nc = _CACHE.get(key)
    if nc is None:
        nc = build_program(pairs)
        _CACHE[key] = nc
    res = run_bass_kernel_spmd(nc, in_maps, list(range(NCORES)))
    out = np.concatenate(
        [res.results[c]["out"].reshape(RPC, W, D) for c in range(NCORES)], axis=0
    ).astype(np.float32)
    # invalid rows were never written by the device; zero them here
    st = x_bert_offset[..., 0].astype(np.int64)
    ed = x_bert_offset[..., 1].astype(np.int64)
    valid = (np.asarray(x_mask) > 0) & (ed > st)
    out[~valid] = 0.0
    return out
